# Initial kernel scaffold
#
"""Per-channel 5x5 local attention (sparse_attention) on 8 Trainium2 cores.

Sharding: data-parallel over batch B=8 (one image per NeuronCore).

Per-core layout:
  partition p = half*64 + c   (half = top/bottom 32 image rows, c = channel)
  free dim    = padded pixels of that half: 36 rows x 68 cols = 2448

Engine-balanced pipeline per window position p=(dh,dw):
  l   = (k_shift + bias[c,p])*q    DVE scalar_tensor_tensor     fp32
  E   = exp(l)                     ACT (bf16 out)
  S  += E                          PE identity-matmul accum     bf16 in, fp32 PSUM
  M   = E * v_shift                DVE tensor_tensor (2x bf16)
  O  += M                          PE identity-matmul accum
  out = O * recip(S)               DVE                          fp32

The 25-way softmax reductions ride the tensor engine (exact fp32 PSUM
accumulation of bf16-quantized terms); projections are exact fp32 matmuls.
"""

import numpy as np
from contextlib import ExitStack

import concourse.bacc as bacc
import concourse.tile as tile
from concourse import mybir
from concourse.bass_utils import run_bass_kernel_spmd

F32 = mybir.dt.float32
BF16 = mybir.dt.bfloat16
F16 = mybir.dt.float16

K = 5
PAD = 2
B, C, H, W = 8, 64, 64, 64
HALF = H // 2            # 32 image rows per half
PR = HALF + 2 * PAD      # 36 padded rows stored per half
PC = W + 2 * PAD         # 68 padded cols
NPAD = PR * PC           # 2448
NPIX = HALF * W          # 2048 pixels per partition
P25 = K * K
MM_N = 512               # matmul free-dim tile (one PSUM bank fp32)
N_BANKS = NPIX // MM_N   # 4



def _build_program():
    nc = bacc.Bacc("TRN2", target_bir_lowering=False)

    xp_d = nc.dram_tensor("xp", [128, NPAD], F32, kind="ExternalInput")
    w3_d = nc.dram_tensor("w3", [128, 3 * 128], F32, kind="ExternalInput")
    bias_d = nc.dram_tensor("bias", [128, P25], F32, kind="ExternalInput")
    id_d = nc.dram_tensor("ident", [128, 128], F32, kind="ExternalInput")
    out_d = nc.dram_tensor("out", [128, NPIX], F32, kind="ExternalOutput")

    with tile.TileContext(nc) as tc, ExitStack() as ctx:
        const = ctx.enter_context(tc.tile_pool(name="const", bufs=1))
        pipe = ctx.enter_context(tc.tile_pool(name="pipe", bufs=8))
        outp = ctx.enter_context(tc.tile_pool(name="outp", bufs=2))

        w3 = const.tile([128, 3, 128], F32)
        nc.sync.dma_start(out=w3, in_=w3_d[:, :].rearrange("p (t m) -> p t m", t=3))
        bias_sb = const.tile([128, P25], F32)
        nc.sync.dma_start(out=bias_sb, in_=bias_d[:, :])
        id32 = const.tile([128, 128], F32)
        nc.sync.dma_start(out=id32, in_=id_d[:, :])
        ident = const.tile([128, 128], BF16)
        nc.vector.tensor_copy(out=ident, in_=id32)

        # ---- projections q/k/v (PE, exact fp32) ----
        with tc.tile_pool(name="xload", bufs=1) as xpool, \
             tc.tile_pool(name="mm", bufs=8, space="PSUM") as mmp:
            xp = xpool.tile([128, NPAD], F32)
            proj = {}
            edges = list(range(0, NPAD, MM_N)) + [NPAD]
            for c0, c1 in zip(edges[:-1], edges[1:]):
                nc.sync.dma_start(out=xp[:, c0:c1], in_=xp_d[:, c0:c1])
            xp3 = xp.rearrange("p (r c) -> p r c", r=PR)
            qd = const.tile([128, NPIX], F16, tag="qd")
            for ti, name in ((1, "k"), (0, "q"), (2, "v")):
                if name == "q":
                    # project only the interior pixels, straight into the
                    # dense q layout (strided moving operand)
                    for j in range(4):
                        ps = mmp.tile([128, MM_N], F32, tag="ps")
                        nc.tensor.matmul(
                            out=ps,
                            lhsT=w3[:, ti, :],
                            rhs=xp3[:, PAD + 8 * j : PAD + 8 * j + 8, PAD : PAD + W],
                            start=True,
                            stop=True,
                        )
                        nc.scalar.copy(
                            out=qd[:, MM_N * j : MM_N * (j + 1)], in_=ps
                        )
                    continue
                if name == "v":
                    # v is only consumed in bf16; k/q only in fp16 — the
                    # evacuation copy does the cast
                    dst = const.tile([128, NPAD], BF16, tag="vb")
                else:
                    dst = const.tile([128, NPAD], F16, tag=f"{name}pad")
                for c0, c1 in zip(edges[:-1], edges[1:]):
                    ps = mmp.tile([128, MM_N], F32, tag="ps")
                    nc.tensor.matmul(
                        out=ps[:, : c1 - c0],
                        lhsT=w3[:, ti, :],
                        rhs=xp[:, c0:c1],
                        start=True,
                        stop=True,
                    )
                    nc.scalar.copy(out=dst[:, c0:c1], in_=ps[:, : c1 - c0])
                proj[name] = dst

        k3 = proj["k"].rearrange("p (r c) -> p r c", r=PR)

        # odd-shift copies to keep 16-bit packing alignment for odd dw
        k1 = const.tile([128, NPAD], F16, tag="k1")
        nc.scalar.copy(out=k1[:, : NPAD - 1], in_=proj["k"][:, 1:])
        k13 = k1.rearrange("p (r c) -> p r c", r=PR)
        vb = proj["v"]
        vb1 = const.tile([128, NPAD], BF16, tag="vb1")
        nc.scalar.copy(out=vb1[:, : NPAD - 1], in_=vb[:, 1:])
        vb3 = vb.rearrange("p (r c) -> p r c", r=PR)
        vb13 = vb1.rearrange("p (r c) -> p r c", r=PR)

        # ---- attention: stream one window position at a time ----
        acc = ctx.enter_context(tc.tile_pool(name="acc", bufs=1, space="PSUM"))
        S_ps = acc.tile([128, NPIX], F32)  # 4 PSUM banks
        O_ps = acc.tile([128, NPIX], F32)  # 4 PSUM banks

        p_order = [p for p in range(P25) if p % K in (0, 2, 4)] + \
                  [p for p in range(P25) if p % K in (1, 3)]
        for pi, p in enumerate(p_order):
            dh, dw = divmod(p, K)
            if dw % 2 == 0:
                kslice = k3[:, dh : dh + HALF, dw : dw + W]
            else:
                kslice = k13[:, dh : dh + HALF, dw - 1 : dw - 1 + W]
            kb = pipe.tile([128, NPIX], F16, tag="kb")
            nc.vector.tensor_scalar_add(
                out=kb, in0=kslice, scalar1=bias_sb[:, p : p + 1]
            )
            t_log = pipe.tile([128, NPIX], F16, tag="log")
            nc.vector.tensor_mul(out=t_log, in0=kb, in1=qd)

            t_e = pipe.tile([128, NPIX], BF16, tag="E")
            nc.scalar.activation(
                out=t_e, in_=t_log, func=mybir.ActivationFunctionType.Exp
            )

            if dw % 2 == 0:
                vslice = vb3[:, dh : dh + HALF, dw : dw + W]
            else:
                vslice = vb13[:, dh : dh + HALF, dw - 1 : dw - 1 + W]
            t_m = pipe.tile([128, NPIX], BF16, tag="M")
            nc.vector.tensor_mul(out=t_m, in0=t_e, in1=vslice)

            for j in range(N_BANKS):
                sl = slice(j * MM_N, (j + 1) * MM_N)
                nc.tensor.matmul(
                    out=S_ps[:, sl],
                    lhsT=ident,
                    rhs=t_e[:, sl],
                    start=(pi == 0),
                    stop=(pi == P25 - 1),
                )
                nc.tensor.matmul(
                    out=O_ps[:, sl],
                    lhsT=ident,
                    rhs=t_m[:, sl],
                    start=(pi == 0),
                    stop=(pi == P25 - 1),
                )

        # ---- out = O / S  (per-bank so DMA overlaps the arithmetic) ----
        for j in range(N_BANKS):
            sl = slice(j * MM_N, (j + 1) * MM_N)
            r_sb = outp.tile([128, MM_N], F32, tag="R")
            nc.vector.reciprocal_approx_fast(out=r_sb, in_=S_ps[:, sl])
            ob = outp.tile([128, MM_N], F32, tag="ob")
            nc.vector.tensor_mul(out=ob, in0=O_ps[:, sl], in1=r_sb)
            nc.sync.dma_start(out=out_d[:, sl], in_=ob)

    nc.finalize()
    return nc


_NC_CACHE = {}


def _get_nc():
    if "nc" not in _NC_CACHE:
        _NC_CACHE["nc"] = _build_program()
    return _NC_CACHE["nc"]


def _host_prep(x, Wq, Wk, Wv, rel_h, rel_w):
    x = np.asarray(x, np.float32)
    bias = np.concatenate(
        [
            np.broadcast_to(np.asarray(rel_h, np.float32), (C // 2, K, K)),
            np.broadcast_to(np.asarray(rel_w, np.float32), (C // 2, K, K)),
        ],
        axis=0,
    ).reshape(C, P25)
    bias128 = np.ascontiguousarray(np.tile(bias, (2, 1)))  # [128, 25]

    w3 = np.zeros((128, 3, 128), np.float32)
    for ti, Wt in enumerate((Wq, Wk, Wv)):
        lhsT = np.asarray(Wt, np.float32).T  # lhsT[c_in, c_out]
        w3[0:64, ti, 0:64] = lhsT
        w3[64:128, ti, 64:128] = lhsT
    w3 = np.ascontiguousarray(w3.reshape(128, 3 * 128))

    ident = np.eye(128, dtype=np.float32)

    xpad = np.pad(x, ((0, 0), (0, 0), (PAD, PAD), (PAD, PAD)))  # [8,64,68,68]
    in_maps = []
    for b in range(B):
        top = xpad[b, :, 0:PR, :]
        bot = xpad[b, :, HALF : HALF + PR, :]
        xp = np.ascontiguousarray(
            np.concatenate([top, bot], axis=0).reshape(128, NPAD)
        )
        in_maps.append({"xp": xp, "w3": w3, "bias": bias128, "ident": ident})
    return in_maps


def _host_gather(results):
    out = np.empty((B, C, H, W), np.float32)
    for b, r in enumerate(results):
        ob = np.asarray(r["out"]).astype(np.float32).reshape(2, C, HALF, W)
        out[b] = np.concatenate([ob[0], ob[1]], axis=1)
    return out


def kernel(**inputs) -> np.ndarray:
    nc = _get_nc()
    in_maps = _host_prep(
        inputs["x"], inputs["Wq"], inputs["Wk"], inputs["Wv"],
        inputs["rel_h"], inputs["rel_w"],
    )
    res = run_bass_kernel_spmd(nc, in_maps, core_ids=list(range(8)))
    return _host_gather(res.results)



# revision 15
# speedup vs baseline: 1.2350x; 1.2350x over previous
"""Per-channel 5x5 local attention (sparse_attention) on 8 Trainium2 cores.

Sharding: data-parallel over batch B=8 (one image per NeuronCore).

Per-core layout:
  partition p = half*64 + c   (half = top/bottom 32 image rows, c = channel)
  free dim    = padded pixels of that half: 36 rows x 68 cols = 2448

Per-position pipeline, balancing DVE and ACT at ~77us each (GPSIMD's
stock elementwise ucode measured 29us/tile = unusable, so everything
rides these two engines plus the PE accumulator):
  kb  = k_shift + bias[c,p]   even dw (15): DVE tensor_scalar (4x-ish)
                              odd dw (10): ACT Identity-with-bias (also
                              dodges the DVE 4B-alignment rule on the
                              odd-offset strided k read)
  l   = kb * q                DVE tensor_tensor   f16, 2x mode
  E   = exp(l)                ACT                 bf16 out
  M   = E * v_shift           DVE tensor_tensor   bf16, 2x mode
  S  += E ; O += M            PE identity-matmul  fp32 PSUM accum
  out = O * recip(S)          DVE

Software pipelining: ACT biases lead by 2 positions, exp lags the logit
by 1, M/O lag by 2 — every stream stays dense. Ramp: params DMA first,
x chunks spread over the three DMA-capable queues, dummy matmuls warm
the PE HAM clock gate during the DMA wait.
"""

import numpy as np
from contextlib import ExitStack

import concourse.bacc as bacc
import concourse.tile as tile
from concourse import mybir
from concourse.bass_utils import run_bass_kernel_spmd

F32 = mybir.dt.float32
BF16 = mybir.dt.bfloat16
F16 = mybir.dt.float16

K = 5
PAD = 2
B, C, H, W = 8, 64, 64, 64
HALF = H // 2            # 32 image rows per half
PR = HALF + 2 * PAD      # 36 padded rows stored per half
PC = W + 2 * PAD         # 68 padded cols
NPAD = PR * PC           # 2448
NPIX = HALF * W          # 2048 pixels per partition
P25 = K * K
MM_N = 512               # matmul free-dim tile (one PSUM bank fp32)
N_BANKS = NPIX // MM_N   # 4
N_WARM = 14              # dummy matmuls to lift the PE HAM clock gate

# positions whose bias-add runs on ACT (odd dw)
ACT_BIAS = {p for p in range(P25) if p % K in (1, 3)}


def _build_program():
    nc = bacc.Bacc("TRN2", target_bir_lowering=False)

    xp_d = nc.dram_tensor("xp", [128, NPAD], F32, kind="ExternalInput")
    w3_d = nc.dram_tensor("w3", [128, 3 * 128], F32, kind="ExternalInput")
    bias_d = nc.dram_tensor("bias", [128, P25], F32, kind="ExternalInput")
    id_d = nc.dram_tensor("ident", [128, 128], F32, kind="ExternalInput")
    out_d = nc.dram_tensor("out", [128, NPIX], F32, kind="ExternalOutput")

    with tile.TileContext(nc) as tc, ExitStack() as ctx:
        const = ctx.enter_context(tc.tile_pool(name="const", bufs=1))
        kbp = ctx.enter_context(tc.tile_pool(name="kbp", bufs=8))
        tlp = ctx.enter_context(tc.tile_pool(name="tlp", bufs=5))
        ep = ctx.enter_context(tc.tile_pool(name="ep", bufs=6))
        mp = ctx.enter_context(tc.tile_pool(name="mp", bufs=5))
        outp = ctx.enter_context(tc.tile_pool(name="outp", bufs=2))

        warm = const.tile([128, MM_N], BF16)
        nc.gpsimd.memset(warm, 0.0)

        w3 = const.tile([128, 3, 128], F32)
        bias_sb = const.tile([128, P25], F32)
        id32 = const.tile([128, 128], F32)

        qd = const.tile([128, NPIX], F16, tag="qd")
        kpad = const.tile([128, NPAD], F16, tag="kpad")
        vb = const.tile([128, NPAD], BF16, tag="vb")
        vb1 = const.tile([128, NPAD], BF16, tag="vb1")
        ob = const.tile([128, NPIX], F32, tag="ob")

        k3 = kpad.rearrange("p (r c) -> p r c", r=PR)
        vb3 = vb.rearrange("p (r c) -> p r c", r=PR)
        vb13 = vb1.rearrange("p (r c) -> p r c", r=PR)

        kb_tiles, tl_tiles, e_tiles = {}, {}, {}

        def em_bias(p):
            dh, dw = divmod(p, K)
            kb = kbp.tile([128, NPIX], F16, tag="kb")
            ks = k3[:, dh : dh + HALF, dw : dw + W]
            if p in ACT_BIAS:
                nc.scalar.activation(
                    out=kb, in_=ks,
                    func=mybir.ActivationFunctionType.Identity,
                    bias=bias_sb[:, p : p + 1], scale=1.0,
                )
            else:
                nc.vector.tensor_scalar_add(
                    out=kb, in0=ks, scalar1=bias_sb[:, p : p + 1]
                )
            kb_tiles[p] = kb

        # ---- DMA in, warmup, projections ----
        with tc.tile_pool(name="xload", bufs=1) as xpool, \
             tc.tile_pool(name="wps", bufs=1, space="PSUM") as wpsp, \
             tc.tile_pool(name="mm", bufs=7, space="PSUM") as mmp:
            xp = xpool.tile([128, NPAD], F32)
            edges = list(range(0, NPAD, MM_N)) + [NPAD]
            # small params first (tiny transfers), then x chunks, spread
            # over the three DMA-capable engine queues
            nc.sync.dma_start(out=id32, in_=id_d[:, :])
            nc.gpsimd.dma_start(
                out=w3, in_=w3_d[:, :].rearrange("p (t m) -> p t m", t=3)
            )
            nc.gpsimd.dma_start(out=bias_sb, in_=bias_d[:, :])
            issuers = [nc.sync, nc.scalar, nc.gpsimd, nc.sync, nc.scalar]
            for eng, (c0, c1) in zip(issuers, zip(edges[:-1], edges[1:])):
                eng.dma_start(out=xp[:, c0:c1], in_=xp_d[:, c0:c1])
            ident = const.tile([128, 128], BF16)
            nc.vector.tensor_copy(out=ident, in_=id32)

            # dummy matmuls on the memset tile (no input dependency): keep
            # PE busy through the DMA wait so the HAM clock gate opens
            # (1.2 -> 2.4 GHz) before the projections
            wps = wpsp.tile([128, MM_N], F32, tag="warm")
            for _ in range(N_WARM):
                nc.tensor.matmul(out=wps, lhsT=warm[:, :128], rhs=warm,
                                 start=True, stop=True)

            # k first (per-chunk evac on DVE, which is idle early; ACT is
            # freed for the lead biases)
            for c0, c1 in zip(edges[:-1], edges[1:]):
                ps = mmp.tile([128, MM_N], F32, tag="ps")
                nc.tensor.matmul(
                    out=ps[:, : c1 - c0], lhsT=w3[:, 1, :], rhs=xp[:, c0:c1],
                    start=True, stop=True,
                )
                nc.vector.tensor_copy(out=kpad[:, c0:c1], in_=ps[:, : c1 - c0])

            # q straight into dense layout, DVE evacuates (DVE is idle early)
            xp3 = xp.rearrange("p (r c) -> p r c", r=PR)
            for j in range(4):
                ps = mmp.tile([128, MM_N], F32, tag="ps")
                nc.tensor.matmul(
                    out=ps,
                    lhsT=w3[:, 0, :],
                    rhs=xp3[:, PAD + 8 * j : PAD + 8 * j + 8, PAD : PAD + W],
                    start=True, stop=True,
                )
                nc.vector.tensor_copy(out=qd[:, MM_N * j : MM_N * (j + 1)], in_=ps)

            # ACT leads with the first odd-dw biases while v projects
            em_bias(1)
            em_bias(3)

            for c0, c1 in zip(edges[:-1], edges[1:]):
                ps = mmp.tile([128, MM_N], F32, tag="ps")
                nc.tensor.matmul(
                    out=ps[:, : c1 - c0], lhsT=w3[:, 2, :], rhs=xp[:, c0:c1],
                    start=True, stop=True,
                )
                nc.scalar.copy(out=vb[:, c0:c1], in_=ps[:, : c1 - c0])
            # odd-shift copy keeps 16-bit packing alignment for odd-dw M;
            # same-dtype copy, so the (otherwise idle) DMA engine does it
            nc.gpsimd.dma_start(out=vb1[:, : NPAD - 1], in_=vb[:, 1:])

        # ---- attention: per-position software pipeline ----
        acc = ctx.enter_context(tc.tile_pool(name="acc", bufs=1, space="PSUM"))
        S_ps = acc.tile([128, NPIX], F32)  # 4 PSUM banks
        O_ps = acc.tile([128, NPIX], F32)  # 4 PSUM banks

        def em_logit(p):
            tl = tlp.tile([128, NPIX], F16, tag="log")
            nc.vector.tensor_mul(out=tl, in0=kb_tiles.pop(p), in1=qd)
            tl_tiles[p] = tl

        def em_exp(p):
            t_e = ep.tile([128, NPIX], BF16, tag="E")
            nc.scalar.activation(
                out=t_e, in_=tl_tiles.pop(p),
                func=mybir.ActivationFunctionType.Exp,
            )
            e_tiles[p] = t_e
            for j in range(N_BANKS):
                sl = slice(j * MM_N, (j + 1) * MM_N)
                nc.tensor.matmul(
                    out=S_ps[:, sl], lhsT=ident, rhs=t_e[:, sl],
                    start=(p == 0), stop=(p == P25 - 1),
                )

        def em_mul(p):
            dh, dw = divmod(p, K)
            if dw % 2 == 0:
                vs = vb3[:, dh : dh + HALF, dw : dw + W]
            else:
                vs = vb13[:, dh : dh + HALF, dw - 1 : dw - 1 + W]
            t_m = mp.tile([128, NPIX], BF16, tag="M")
            nc.vector.tensor_mul(out=t_m, in0=e_tiles.pop(p), in1=vs)
            for j in range(N_BANKS):
                sl = slice(j * MM_N, (j + 1) * MM_N)
                nc.tensor.matmul(
                    out=O_ps[:, sl], lhsT=ident, rhs=t_m[:, sl],
                    start=(p == 0), stop=(p == P25 - 1),
                )

        for t in range(P25 + 3):
            if t + 2 < P25 and (t + 2) in ACT_BIAS and t + 2 > 3:
                em_bias(t + 2)          # ACT works 2 positions ahead
            if t < P25:
                if t not in ACT_BIAS:
                    em_bias(t)          # DVE bias just-in-time
                em_logit(t)
            if 0 <= t - 1 < P25:
                em_exp(t - 1)
            if 0 <= t - 3 < P25:
                em_mul(t - 3)

        # ---- out = O / S; two output DMAs on separate queues ----
        for j in range(N_BANKS):
            sl = slice(j * MM_N, (j + 1) * MM_N)
            r_sb = outp.tile([128, MM_N], F32, tag="R")
            nc.vector.reciprocal_approx_fast(out=r_sb, in_=S_ps[:, sl])
            nc.vector.tensor_mul(out=ob[:, sl], in0=O_ps[:, sl], in1=r_sb)
            if j == 0:
                nc.sync.dma_start(out=out_d[:, :MM_N], in_=ob[:, :MM_N])
            elif j == 1:
                nc.scalar.dma_start(
                    out=out_d[:, MM_N : 2 * MM_N], in_=ob[:, MM_N : 2 * MM_N])
            elif j == 2:
                nc.gpsimd.dma_start(
                    out=out_d[:, 2 * MM_N : 3 * MM_N],
                    in_=ob[:, 2 * MM_N : 3 * MM_N])
        nc.sync.dma_start(out=out_d[:, 3 * MM_N :], in_=ob[:, 3 * MM_N :])

    nc.finalize()
    return nc


_NC_CACHE = {}


def _get_nc():
    if "nc" not in _NC_CACHE:
        _NC_CACHE["nc"] = _build_program()
    return _NC_CACHE["nc"]


def _host_prep(x, Wq, Wk, Wv, rel_h, rel_w):
    x = np.asarray(x, np.float32)
    bias = np.concatenate(
        [
            np.broadcast_to(np.asarray(rel_h, np.float32), (C // 2, K, K)),
            np.broadcast_to(np.asarray(rel_w, np.float32), (C // 2, K, K)),
        ],
        axis=0,
    ).reshape(C, P25)
    bias128 = np.ascontiguousarray(np.tile(bias, (2, 1)))  # [128, 25]

    w3 = np.zeros((128, 3, 128), np.float32)
    for ti, Wt in enumerate((Wq, Wk, Wv)):
        lhsT = np.asarray(Wt, np.float32).T  # lhsT[c_in, c_out]
        w3[0:64, ti, 0:64] = lhsT
        w3[64:128, ti, 64:128] = lhsT
    w3 = np.ascontiguousarray(w3.reshape(128, 3 * 128))

    ident = np.eye(128, dtype=np.float32)

    xpad = np.pad(x, ((0, 0), (0, 0), (PAD, PAD), (PAD, PAD)))  # [8,64,68,68]
    in_maps = []
    for b in range(B):
        top = xpad[b, :, 0:PR, :]
        bot = xpad[b, :, HALF : HALF + PR, :]
        xp = np.ascontiguousarray(
            np.concatenate([top, bot], axis=0).reshape(128, NPAD)
        )
        in_maps.append({"xp": xp, "w3": w3, "bias": bias128, "ident": ident})
    return in_maps


def _host_gather(results):
    out = np.empty((B, C, H, W), np.float32)
    for b, r in enumerate(results):
        ob = np.asarray(r["out"]).astype(np.float32).reshape(2, C, HALF, W)
        out[b] = np.concatenate([ob[0], ob[1]], axis=1)
    return out


def kernel(**inputs) -> np.ndarray:
    nc = _get_nc()
    in_maps = _host_prep(
        inputs["x"], inputs["Wq"], inputs["Wk"], inputs["Wv"],
        inputs["rel_h"], inputs["rel_w"],
    )
    res = run_bass_kernel_spmd(nc, in_maps, core_ids=list(range(8)))
    return _host_gather(res.results)
